# revision 25
# baseline (speedup 1.0000x reference)
"""LocationAwareAttention Trainium2 kernel (8-core batch-parallel, bf16).

Math (per batch row b):
  conv  = conv1d(last_attn, conv_w, pad 1) + conv_b          # (L, H)
  c     = query@Wq.T + value@Wv.T + conv + bias              # (L, H)
  score = tanh(c) @ Ws + bs                                  # (L,)
  sig   = sigmoid(score); attn = sig / sum(sig)
  out   = [attn @ value, query]

Device strategy: batch-shard over 8 cores (4 rows each). One streaming pass
over value per layout (natural tiles for the context matmul, xbar-transposed
tiles for the Wv matmul). tanh(x) is computed as 2*sigmoid(2x)-1 with the
affine pieces folded into host-side weight scaling, so the scalar engine only
ever needs the sigmoid table set. The smoothing normalization is deferred:
unnormalized sig-weighted value sums and sum(sig) are accumulated and divided
once at the end.
"""

import sys

for _p in ("/opt/trn_rl_repo",):
    if _p not in sys.path:
        sys.path.insert(0, _p)

import numpy as np
import ml_dtypes

BF16 = ml_dtypes.bfloat16

B, L, H = 32, 4096, 512
NCORES = 8
BL = B // NCORES          # 4 batch rows per core
P = 128
LB = 512                  # l-block
NLB = L // LB             # 8 blocks
HC = H // P               # 4 contraction chunks
OC = H // P               # 4 output chunks
LT = LB // P              # 4 l-subtiles per block

_CACHE = {}


def _build():
    import concourse.bass as bass
    import concourse.mybir as mybir
    from concourse import bacc
    from concourse.tile import TileContext
    from concourse.masks import make_identity

    dt = mybir.dt
    f32, bf16 = dt.float32, dt.bfloat16
    SIG = mybir.ActivationFunctionType.Sigmoid
    ts = bass.ts

    nc = bacc.Bacc(
        "TRN2",
        target_bir_lowering=False,
        debug=False,
        enable_asserts=False,
        num_devices=NCORES,
    )

    value_d = nc.dram_tensor("value_bf", [BL, L, H], bf16, kind="ExternalInput").ap()
    query_d = nc.dram_tensor("query", [BL, H], f32, kind="ExternalInput").ap()
    qpb_d = nc.dram_tensor("qpb2", [P, OC * BL], f32, kind="ExternalInput").ap()
    aug_d = nc.dram_tensor("last_aug", [BL, 3, L], bf16, kind="ExternalInput").ap()
    wvT_d = nc.dram_tensor("wvT2", [H, H], bf16, kind="ExternalInput").ap()
    cwT_d = nc.dram_tensor("convwT2", [3, H], bf16, kind="ExternalInput").ap()
    ws_d = nc.dram_tensor("ws2", [P, OC], bf16, kind="ExternalInput").ap()
    bs_d = nc.dram_tensor("bs_col", [P, 1], f32, kind="ExternalInput").ap()
    out_d = nc.dram_tensor("out", [BL, 2 * H], f32, kind="ExternalOutput").ap()
    attn_d = nc.dram_tensor("attn", [BL, L], f32, kind="ExternalOutput").ap()

    # raw SBUF allocations must happen before TileContext so the pool
    # allocator sees them as reserved
    ident = nc.alloc_sbuf_tensor("ident_sb", [P, P], f32).ap()
    wv_sb = [nc.alloc_sbuf_tensor(f"wv_sb{hc}", [P, H], bf16).ap() for hc in range(HC)]
    cw_full = nc.alloc_sbuf_tensor("cw_sb", [P, H], bf16).ap()
    cw_sb = cw_full[0:3, :]
    ws_sb = nc.alloc_sbuf_tensor("ws_sb", [P, OC], bf16).ap()
    bs_sb = nc.alloc_sbuf_tensor("bs_sb", [P, 1], f32).ap()
    aug_sb = [nc.alloc_sbuf_tensor(f"aug_sb{b}", [P, L], bf16).ap()[0:3, :] for b in range(BL)]
    sig_rows = nc.alloc_sbuf_tensor("sig_rows", [P, L], f32).ap()
    out_sb = nc.alloc_sbuf_tensor("out_sb", [P, 2 * H], f32).ap()
    qpb_sb = nc.alloc_sbuf_tensor("qpb_sb", [P, OC * BL], f32).ap()
    s_col = nc.alloc_sbuf_tensor("s_col", [P, 1], f32).ap()
    sinv = nc.alloc_sbuf_tensor("sinv", [P, 1], f32).ap()

    with TileContext(nc) as tc:
        with (
            tc.tile_pool(name="valn", bufs=2) as valnp,
            tc.tile_pool(name="vt", bufs=3) as vtp,
            tc.tile_pool(name="et", bufs=3) as etp,
            tc.tile_pool(name="sigc", bufs=4) as sigcp,
            tc.tile_pool(name="ps_ct", bufs=4, space="PSUM") as ps_ct,
            tc.tile_pool(name="ps_score", bufs=1, space="PSUM") as ps_scorep,
            tc.tile_pool(name="ps_ctx", bufs=1, space="PSUM") as ps_ctxp,
            tc.tile_pool(name="ps_small", bufs=2, space="PSUM") as ps_small,
        ):
            # ---------------- constants / setup ----------------
            make_identity(nc, ident)
            for hc in range(HC):
                nc.scalar.dma_start(wv_sb[hc], wvT_d[ts(hc, P), :])
            nc.scalar.dma_start(cw_sb, cwT_d)
            nc.scalar.dma_start(ws_sb, ws_d)
            nc.scalar.dma_start(bs_sb, bs_d)
            for b in range(BL):
                nc.scalar.dma_start(aug_sb[b], aug_d[b, :, :])
            nc.gpsimd.memset(sig_rows, 0.0)
            for b in range(BL):
                nc.scalar.dma_start(out_sb[32 * b : 32 * b + 1, H:], query_d[b : b + 1, :])
            nc.scalar.dma_start(qpb_sb, qpb_d)

            # ctx accumulator: row 32*b holds unnormalized context for batch b
            ctx_ps = ps_ctxp.tile([P, H], f32, tag="ctx")

            # ---------------- streaming main loop ----------------
            for lb in range(NLB):
                l0 = lb * LB
                score_ps = ps_scorep.tile([P, LB], f32, tag="score")
                valn_tiles = []
                for b in range(BL):
                    valn = valnp.tile([P, LT * H], bf16, tag=f"valn{b}")
                    nc.scalar.dma_start(
                        valn[:].rearrange("p (lt h) -> p lt h", lt=LT),
                        value_d[b, l0 : l0 + LB, :].rearrange("(lt p) h -> p lt h", p=P),
                    )
                    valn_tiles.append(valn)
                    vts = []
                    for hc in range(HC):
                        vt = vtp.tile([P, LB], bf16, tag=f"vt{hc}")
                        nc.sync.dma_start_transpose(
                            vt, value_d[b, l0 : l0 + LB, ts(hc, P)]
                        )
                        vts.append(vt)
                    et = etp.tile([P, OC * LB], bf16, tag="et")
                    for oc in range(OC):
                        ct = ps_ct.tile([P, LB], f32, tag="ct")
                        for hc in range(HC):
                            nc.tensor.matmul(
                                ct,
                                wv_sb[hc][:, ts(oc, P)],
                                vts[hc],
                                start=(hc == 0),
                                stop=False,
                            )
                        nc.tensor.matmul(
                            ct,
                            cw_sb[0:3, ts(oc, P)],
                            aug_sb[b][0:3, l0 : l0 + LB],
                            start=False,
                            stop=True,
                        )
                        # tanh(x) = 2*sigmoid(2x) - 1; weights pre-scaled by 2
                        nc.scalar.activation(
                            et[:, ts(oc, LB)],
                            ct,
                            SIG,
                            bias=qpb_sb[:, oc * BL + b : oc * BL + b + 1],
                            scale=1.0,
                        )
                        nc.tensor.matmul(
                            score_ps[32 * b : 32 * b + 1, :],
                            ws_sb[:, oc : oc + 1],
                            et[:, ts(oc, LB)],
                            start=(oc == 0),
                            stop=(oc == OC - 1),
                            skip_group_check=True,
                            tile_position=(0, 32 * b),
                        )
                    nc.scalar.activation(
                        sig_rows[32 * b : 32 * b + 1, l0 : l0 + LB],
                        score_ps[32 * b : 32 * b + 1, :],
                        SIG,
                        bias=bs_sb[32 * b : 32 * b + 1, :],
                        scale=1.0,
                    )
                # sig rows -> columns (PE transpose), then context accumulation
                for g in range(LT):
                    st_ps = ps_small.tile([P, P], f32, tag="small")
                    nc.tensor.transpose(
                        st_ps, sig_rows[:, l0 + g * P : l0 + (g + 1) * P], ident
                    )
                    sigcol = sigcp.tile([P, P], bf16, tag="sigcol")
                    nc.vector.tensor_copy(sigcol, st_ps)
                    for b in range(BL):
                        nc.tensor.matmul(
                            ctx_ps[32 * b : 32 * b + 1, :],
                            sigcol[:, 32 * b : 32 * b + 1],
                            valn_tiles[b][:, ts(g, H)],
                            start=(lb == 0 and g == 0),
                            stop=(lb == NLB - 1 and g == LT - 1),
                            skip_group_check=True,
                            tile_position=(0, 32 * b),
                        )

            # ---------------- finalize ----------------
            nc.vector.tensor_reduce(
                s_col, sig_rows, axis=mybir.AxisListType.X, op=mybir.AluOpType.add
            )
            nc.vector.reciprocal(sinv, s_col)
            nc.vector.tensor_scalar_mul(sig_rows, sig_rows, sinv)
            for b in range(BL):
                nc.vector.tensor_scalar_mul(
                    out_sb[32 * b : 32 * b + 1, 0:H],
                    ctx_ps[32 * b : 32 * b + 1, :],
                    sinv[32 * b : 32 * b + 1, :],
                )
            for b in range(BL):
                nc.scalar.dma_start(attn_d[b : b + 1, :], sig_rows[32 * b : 32 * b + 1, :])
                nc.scalar.dma_start(out_d[b : b + 1, :], out_sb[32 * b : 32 * b + 1, :])

    nc.compile()
    return nc


def _get_nc():
    if "nc" not in _CACHE:
        _CACHE["nc"] = _build()
    return _CACHE["nc"]


def prep_in_maps(query, value, last_attn, conv_w, conv_b, Wq, Wv, Ws, bs, bias):
    query = np.asarray(query, np.float32)
    value = np.asarray(value, np.float32)
    last_attn = np.asarray(last_attn, np.float32)
    conv_w = np.asarray(conv_w, np.float32)
    conv_b = np.asarray(conv_b, np.float32)
    Wq = np.asarray(Wq, np.float32)
    Wv = np.asarray(Wv, np.float32)
    Ws = np.asarray(Ws, np.float32)
    bias = np.asarray(bias, np.float32)
    bs_f = float(np.asarray(bs, np.float32))

    wvT2 = np.ascontiguousarray((2.0 * Wv.T)).astype(BF16)
    cwT2 = np.ascontiguousarray((2.0 * conv_w[:, 0, :].T)).astype(BF16)  # [3, H]
    ws2 = np.ascontiguousarray((2.0 * Ws).reshape(OC, P).T).astype(BF16)
    bs_col = np.full((P, 1), bs_f - float(Ws.sum()), np.float32)

    # qpb2[b] = 2*(query[b] @ Wq.T + bias + conv_b), laid out [P, oc*BL+b]
    qall = query[:, 0, :] @ Wq.T + (bias + conv_b)[None, :]
    qall = (2.0 * qall).astype(np.float32)  # [B, H]

    pad = np.zeros((B, L + 2), np.float32)
    pad[:, 1 : L + 1] = last_attn
    in_maps = []
    for c in range(NCORES):
        sl = slice(c * BL, (c + 1) * BL)
        aug = np.stack([pad[sl, k : k + L] for k in range(3)], axis=1).astype(BF16)
        qpb2 = np.ascontiguousarray(
            qall[sl].reshape(BL, OC, P).transpose(2, 1, 0).reshape(P, OC * BL)
        )
        in_maps.append(
            {
                "value_bf": value[sl].astype(BF16),
                "query": np.ascontiguousarray(query[sl, 0, :]),
                "qpb2": qpb2,
                "last_aug": aug,
                "wvT2": wvT2,
                "convwT2": cwT2,
                "ws2": ws2,
                "bs_col": bs_col,
            }
        )
    return in_maps


def kernel(query, value, last_attn, conv_w, conv_b, Wq, Wv, Ws, bs, bias):
    from concourse import bass_utils

    in_maps = prep_in_maps(
        query, value, last_attn, conv_w, conv_b, Wq, Wv, Ws, bs, bias
    )
    nc = _get_nc()
    res = bass_utils.run_bass_kernel_spmd(nc, in_maps, list(range(NCORES)))
    out = np.concatenate([r["out"] for r in res.results], axis=0)
    attn = np.concatenate([r["attn"] for r in res.results], axis=0)
    return out, attn
